# revision 3
# baseline (speedup 1.0000x reference)
"""Trainium2 Bass kernel for the binarized-conv bottleneck block.

Math: out = prelu(prelu(bn3(bconv3(s3))) + x), where
  s1 = binarize(x); c1 = bconv1(s1) (1x1, 128->32)
  s2 = binarize(bn1(c1))  (prelu dropped: it preserves sign)
  c2 = bconv2(s2) (3x3 pad 1, 32->32); s3 = binarize(bn2(c2))
  c3 = bconv3(s3) (1x1, 32->128)

Key choices:
- Binarized values carried as g in {0,1}, s = 2g-1; conv weights are
  2*sign(w), so PSUM gets c' = c_ref + rowsum(sign(w)); rowsum folds into
  per-channel thresholds / the BN3 bias.  3x3 zero-padding is g = 0.5.
  The binary path is exact in f32; only the residual path rounds.
- x and out travel through HBM as bf16 (host casts).  binarize(bf16(x))
  == binarize(x) exactly (same exponent range), and the residual add in
  bf16 costs ~0.4% rel err, far under the 2e-2 gate.  This halves DMA
  traffic: 8.4 MB in + 8.4 MB out per core.
- The image is split into 4 row-bands of 32, one per 32-partition group.
  Bands 1 and 3 are stored vertically flipped (host-side permute), so all
  four groups process rows top-down in lockstep: every elementwise stage
  is a single full-width instruction, and the 3x3 conv taps for flipped
  groups just use ky-reversed weight blocks.  Cross-band halo rows are
  exchanged only at the first and last macro.
- Elementwise work is spread over Vector/Scalar/GpSimd (see CFG).

Sharding: data-parallel over batch, one image per NeuronCore (8 cores).
"""
import numpy as np
import ml_dtypes

import concourse.bass as bass
import concourse.mybir as mybir
from concourse import bacc
from concourse.tile import TileContext
from concourse.bass_utils import run_bass_kernel_spmd

F32 = mybir.dt.float32
BF16 = mybir.dt.bfloat16
FP8 = mybir.dt.float8e4
AF = mybir.ActivationFunctionType
OP = mybir.AluOpType

B, C, CI, H, W = 8, 128, 32, 128, 256
HW = H * W                    # 32768
BH = 32                       # band height (rows per band)
BPX = BH * W                  # 8192 pixels per band
WP = W + 2                    # 258
SLAB = BH + 2                 # 34 rows: halo + 32 interior + halo
MACRO = 2048                  # pixels per macro = 2 rows x 4 bands
NMAC = 16
EPS = 1e-5

_CACHE = {}

# engine assignment: g1/th1/th2/radd/prelu2 -> vector|scalar|gpsimd
CFG = dict(g1="vector", th1="vector", th2="vector", radd="gpsimd",
           prelu2="vector", lag=2, wdt="bf16")


def _eng(nc, name):
    return {"vector": nc.vector, "gpsimd": nc.gpsimd,
            "scalar": nc.scalar}[CFG[name]]


def _build(a3: float, a_out: float, repeat: int = 1):
    WDT = BF16 if CFG.get("wdt", "bf16") == "bf16" else FP8
    nc = bacc.Bacc("TRN2", debug=False)

    x_d = nc.dram_tensor("x", [C, HW], BF16, kind="ExternalInput")
    w1_d = nc.dram_tensor("w1s", [C, CI], WDT, kind="ExternalInput")
    w2_d = nc.dram_tensor("w2s", [C, 9 * CI], WDT, kind="ExternalInput")
    w3_d = nc.dram_tensor("w3s", [C, C], WDT, kind="ExternalInput")
    vec_d = nc.dram_tensor("vecs", [C, 4], F32, kind="ExternalInput")
    out_d = nc.dram_tensor("out", [C, HW], BF16, kind="ExternalOutput")

    with TileContext(nc) as tc:
        with (
            tc.tile_pool(name="const", bufs=1) as cpool,
            tc.tile_pool(name="res", bufs=1) as rpool,
            tc.tile_pool(name="work", bufs=3) as wpool,
            tc.tile_pool(name="eplg", bufs=3) as epool,
            tc.tile_pool(name="ps1", bufs=2, space="PSUM") as ps1,
            tc.tile_pool(name="ps2", bufs=2, space="PSUM") as ps2,
            tc.tile_pool(name="ps3", bufs=2, space="PSUM") as ps3,
        ):
            # ---- constants ----
            w1s = cpool.tile([C, CI], WDT)
            nc.sync.dma_start(out=w1s, in_=w1_d[:, :])
            w2s = cpool.tile([C, 9 * CI], WDT)
            nc.sync.dma_start(out=w2s, in_=w2_d[:, :])
            w3s = cpool.tile([C, C], WDT)
            nc.sync.dma_start(out=w3s, in_=w3_d[:, :])
            vecs = cpool.tile([C, 4], F32)
            nc.sync.dma_start(out=vecs, in_=vec_d[:, :])
            t1v = vecs[:, 0:1]
            t2v = vecs[:, 1:2]
            sc3v = vecs[:, 2:3]
            b3v = vecs[:, 3:4]

            # ---- residents ----
            x_sb = rpool.tile([C, HW], BF16)
            xv4 = x_sb.rearrange("p (b r) -> p b r", b=4)       # [128,4,8192]
            ov4 = out_d[:, :].rearrange("p (b r) -> p b r", b=4)
            xv4d = x_d[:, :].rearrange("p (b r) -> p b r", b=4)
            g2b = rpool.tile([128, SLAB * WP], WDT)             # band slabs
            g2b3 = g2b.rearrange("p (r c) -> p r c", c=WP)

            # static borders: left/right pad cols; top rows of bands 0,3
            nc.vector.memset(g2b3[:, :, 0:1], 0.5)
            nc.vector.memset(g2b3[:, :, WP - 1:WP], 0.5)
            nc.vector.memset(g2b3[0:CI, 0:1, :], 0.5)            # band0 top
            nc.vector.memset(g2b3[96:128, 0:1, :], 0.5)          # band3 top

            def halo(src_g, src_row, dst_g, dst_row):
                nc.sync.dma_start(
                    out=g2b3[CI * dst_g:CI * (dst_g + 1), dst_row:dst_row + 1, :],
                    in_=g2b3[CI * src_g:CI * (src_g + 1), src_row:src_row + 1, :])

            def load(m):          # rows for macros m, m+1 (m even)
                nc.sync.dma_start(
                    out=xv4[:, :, 512 * m:512 * (m + 2)],
                    in_=xv4d[:, :, 512 * m:512 * (m + 2)])

            def stage1(m):
                dn = 2 * m           # local row in every band
                g1t = wpool.tile([C, MACRO], WDT, name="g1t")
                _eng(nc, "g1").tensor_scalar(
                    out=g1t.rearrange("p (b r) -> p b r", b=4),
                    in0=xv4[:, :, 512 * m:512 * (m + 1)],
                    scalar1=0.0, scalar2=None, op0=OP.is_gt)
                c1 = ps1.tile([128, 512], F32, name="c1")
                for g in range(4):
                    nc.tensor.matmul(
                        c1[CI * g:CI * (g + 1), :], w1s,
                        g1t[:, 512 * g:512 * (g + 1)], start=True, stop=True,
                        tile_position=(0, CI * g),
                    )
                # threshold -> padded slab rows dn+1, dn+2 (all groups)
                _eng(nc, "th1").tensor_scalar(
                    out=g2b3[:, dn + 1:dn + 3, 1:W + 1],
                    in0=c1, scalar1=t1v, scalar2=None, op0=OP.is_gt)
                if m == 0:
                    halo(2, 1, 1, 0)    # band2 row0 -> band1 top halo
                    halo(1, 1, 2, 0)    # band1 row0 -> band2 top halo
                if m == NMAC - 1:
                    halo(1, 32, 0, 33)  # band1 row31 -> band0 bottom
                    halo(0, 32, 1, 33)  # band0 row31 -> band1 bottom
                    halo(3, 32, 2, 33)  # band3 row31 -> band2 bottom
                    halo(2, 32, 3, 33)  # band2 row31 -> band3 bottom

            def stage23(m):
                dn = 2 * m
                c2 = ps2.tile([128, 512], F32, name="c2")
                for t in range(9):
                    ky, dx = divmod(t, 3)
                    for g in range(4):
                        nc.tensor.matmul(
                            c2[CI * g:CI * (g + 1), :],
                            w2s[CI * g:CI * (g + 1), CI * t:CI * (t + 1)],
                            g2b3[CI * g:CI * (g + 1),
                                 dn + ky:dn + ky + 2, dx:dx + W],
                            start=(t == 0), stop=(t == 8),
                            tile_position=(CI * g, CI * g),
                        )
                s3g = wpool.tile([128, 512], WDT, name="s3g")
                _eng(nc, "th2").tensor_scalar(
                    out=s3g, in0=c2, scalar1=t2v, scalar2=None, op0=OP.is_gt)
                ot = epool.tile([128, 2048], BF16, name="ot")
                for half in range(2):
                    c3 = ps3.tile([128, 1024], F32, name="c3")
                    for jj in range(2):
                        g = 2 * half + jj
                        nc.tensor.matmul(
                            c3[:, 512 * jj:512 * (jj + 1)],
                            w3s[CI * g:CI * (g + 1), :],
                            s3g[CI * g:CI * (g + 1), :],
                            start=True, stop=True, tile_position=(CI * g, 0),
                        )
                    xap = xv4[:, 2 * half:2 * half + 2,
                              512 * m:512 * (m + 1)]
                    p3 = epool.tile([128, 1024], BF16, name="p3")
                    nc.scalar.activation(p3, c3, AF.Prelu, bias=b3v,
                                         scale=sc3v, alpha=a3)
                    rt = epool.tile([128, 1024], BF16, name="rt")
                    _eng(nc, "radd").tensor_tensor(
                        out=rt.rearrange("p (b r) -> p b r", b=2),
                        in0=p3.rearrange("p (b r) -> p b r", b=2),
                        in1=xap, op=OP.add)
                    otv = ot[:, 1024 * half:1024 * (half + 1)]
                    if CFG["prelu2"] == "scalar":
                        nc.scalar.activation(otv, rt, AF.Prelu, alpha=a_out)
                    else:
                        _eng(nc, "prelu2").scalar_tensor_tensor(
                            out=otv, in0=rt, scalar=a_out, in1=rt,
                            op0=OP.mult, op1=OP.max)
                nc.sync.dma_start(
                    out=ov4[:, :, 512 * m:512 * (m + 1)],
                    in_=ot.rearrange("p (b r) -> p b r", b=4))

            LAG = int(CFG.get("lag", 2))

            def whole():
                load(0)
                for m in range(NMAC):
                    if m % 2 == 0 and m + 2 < NMAC:
                        load(m + 2)
                    stage1(m)
                    if m >= LAG:
                        stage23(m - LAG)
                for m in range(NMAC - LAG, NMAC):
                    stage23(m)

            if repeat == 1:
                whole()
            else:
                with tc.For_i(0, repeat, 1):
                    whole()

    nc.compile()
    return nc


def _host_params(w1, g1, b1, m1, v1, w2, g2, b2, m2, v2, w3, g3, b3, m3, v3):
    def sgn(w):
        return np.where(w <= 0, -1.0, 1.0)

    w1 = np.asarray(w1, np.float64).reshape(CI, C)
    w2 = np.asarray(w2, np.float64).reshape(CI, CI, 3, 3)
    w3 = np.asarray(w3, np.float64).reshape(C, CI)
    s1, s2, s3 = sgn(w1), sgn(w2), sgn(w3)

    def bnfold(g, b, m, v):
        inv = np.asarray(g, np.float64) / np.sqrt(np.asarray(v, np.float64) + EPS)
        beta = np.asarray(b, np.float64) - np.asarray(m, np.float64) * inv
        return inv, beta

    inv1, beta1 = bnfold(g1, b1, m1, v1)
    inv2, beta2 = bnfold(g2, b2, m2, v2)
    inv3, beta3 = bnfold(g3, b3, m3, v3)

    wdt = (ml_dtypes.bfloat16 if CFG.get("wdt", "bf16") == "bf16"
           else ml_dtypes.float8_e4m3)
    w1s = (2.0 * s1.T).astype(wdt)                    # [C, CI] lhsT
    # w2s: [128, 9*32]; partitions 32g+c; col block t=(3ky+dx): 2*s2[o,c,ky,dx]
    # groups 1 and 3 process vertically flipped bands -> ky reversed.
    blk = np.zeros((9, CI, CI), np.float64)
    for ky in range(3):
        for dx in range(3):
            blk[3 * ky + dx] = 2.0 * s2[:, :, ky, dx].T   # [c, o]
    w2_fwd = np.concatenate(list(blk), axis=1)            # [32, 288]
    blkf = np.zeros_like(blk)
    for ky in range(3):
        for dx in range(3):
            blkf[3 * ky + dx] = blk[3 * (2 - ky) + dx]
    w2_flip = np.concatenate(list(blkf), axis=1)
    w2st = np.concatenate([w2_fwd, w2_flip, w2_fwd, w2_flip],
                          axis=0).astype(wdt)             # [128, 288]
    w3st = np.tile(2.0 * s3.T, (4, 1)).astype(wdt)        # [32g+c, o]

    rs1 = s1.sum(axis=1)
    rs2 = s2.sum(axis=(1, 2, 3))
    rs3 = s3.sum(axis=1)

    t1 = np.tile(rs1 - beta1 / inv1, 4).astype(np.float32)
    t2 = np.tile(rs2 - beta2 / inv2, 4).astype(np.float32)
    sc3 = inv3.astype(np.float32)
    b3f = (beta3 - inv3 * rs3).astype(np.float32)
    vecs = np.stack([t1, t2, sc3, b3f], axis=1)       # [C, 4] f32
    return w1s, w2st, w3st, vecs


def _permute_in(img):
    """[C, H, W] f32 -> [C, HW] bf16, bands 1,3 vertically flipped."""
    xb = img.reshape(C, 4, BH, W).astype(ml_dtypes.bfloat16)
    xb = np.concatenate(
        [xb[:, 0], xb[:, 1, ::-1], xb[:, 2], xb[:, 3, ::-1]], axis=1)
    return np.ascontiguousarray(xb.reshape(C, HW))


def _permute_out(flat):
    """[C, HW] bf16 (flipped-band layout) -> [C, H, W] f32."""
    o = np.asarray(flat).reshape(C, 4, BH, W).astype(np.float32)
    return np.concatenate(
        [o[:, 0], o[:, 1, ::-1], o[:, 2], o[:, 3, ::-1]], axis=1)


last_results = None


def kernel(**inputs):
    global last_results
    x = np.ascontiguousarray(np.asarray(inputs["x"], np.float32))
    w1s, w2st, w3st, vecs = _host_params(
        inputs["w1"], inputs["g1"], inputs["b1"], inputs["m1"], inputs["v1"],
        inputs["w2"], inputs["g2"], inputs["b2"], inputs["m2"], inputs["v2"],
        inputs["w3"], inputs["g3"], inputs["b3"], inputs["m3"], inputs["v3"])
    a3 = float(np.asarray(inputs["a3"]))
    a_out = float(np.asarray(inputs["a_out"]))

    key = (a3, a_out)
    if key not in _CACHE:
        _CACHE[key] = _build(a3, a_out)
    nc = _CACHE[key]

    shared = {"w1s": w1s, "w2s": w2st, "w3s": w3st, "vecs": vecs}
    in_maps = [dict(x=_permute_in(x[b]), **shared) for b in range(B)]
    res = run_bass_kernel_spmd(nc, in_maps, core_ids=list(range(B)))
    last_results = res
    out = np.stack([_permute_out(res.results[b]["out"]) for b in range(B)])
    return out
